# revision 21
# baseline (speedup 1.0000x reference)
"""Trainium2 Bass kernel for the CCM retrieval problem.

Reference computation (shapes: bs=64, N=1024, D=2048, H=128, C=65):
    z_x   = softmax(cos(all_f, emb)/T/sqrt(N))            [bs, N]
    hf    = head(all_f); hz = head(emb)                   [bs, H], [N, H]
    h1    = relu(BN(hf[b] @ A + b1 + hz[n] @ B))          [bs, N, H]
    y_zx  = softmax((h1 @ mix_w2 + mix_b2) @ clf_w + clf_b)  [bs, N, C]
    p_x   = softmax(sum_n cos(all_f, all_f)/T/sqrt(bs))   [bs]
    out   = z_x @ einsum('b,bnc->nc', p_x, y_zx)          [bs, C]

Device strategy: shard the queue axis N across 8 cores (128 rows each); bs
stays replicated so sum_x is core-local.  Host folds the BN affines into
weights, pre-multiplies mix_w2 @ clf_w (W2C) so the [bs,N,D] intermediate
never exists, pre-normalizes the embedding rows, and precomputes p_x.

Per core: input DMA is spread over four engine queues (sync/vector carry
etf halves, scalar/gpsimd carry w1h halves) so the ~1.3MB payload lands in
half the time; head-layer matmuls are emitted in chunk-arrival order.  The
mixer's first layer collapses to alpha[h,b] (64 cols) and beta[h,n] (128
cols); per b a fused relu(beta + alpha[:,b]) tensor_scalar produces the
logits stationary, spread across DVE (4x mode) / ScalarE / GpSimd one group
ahead of the PE.  exp on ScalarE; the softmax row-sums, reciprocal and
p_x-scale run per 8-b half on DVE so the e2 weighting (split DVE/GpSimd)
and the PSUM-accumulated b-sum start as early as possible.  Each core
returns [64, 66]: columns 0:65 are exp(z-score) @ sum_x partial numerators,
column 65 the z_x softmax denominator partial; the host sums partials over
cores and divides.
"""

import numpy as np
import ml_dtypes

import concourse.bass as bass
import concourse.tile as tile
from concourse import bacc, mybir
from concourse import bass_utils

F32 = mybir.dt.float32
BF16 = mybir.dt.bfloat16
F8 = mybir.dt.float8e4
AX = mybir.AxisListType
ALU = mybir.AluOpType
ACTF = mybir.ActivationFunctionType

T = 0.07
BN_EPS = 1e-5
BS, D, N, H, C = 64, 2048, 1024, 128, 65
NCORES = 8
NLOC = N // NCORES          # 128 queue rows per core
DCH = D // 128              # 16 contraction chunks
CP = 66                     # padded C stride (even -> 4B-aligned bf16 rows)
GRP = 16                    # b's per group (S/e2/accum granularity)
NG = BS // GRP              # 4 groups
HALF = 8                    # b's per 2-bank psum tile (4 per bank)

# tuning knobs --------------------------------------------------------------
# N_WARM_MM: junk matmuls before the head to ramp the PE p-state / HAM.
# SPREADS[g]: (n_dve, n_scalar, n_gpsimd) u-producers for group g; u's for
#   group g are emitted one group ahead of their logits matmuls.
# E2_CHUNKS: (engine, j0, j1) pieces of the e2 = e * w multiply.
import os as _os
N_WARM_MM = int(_os.environ.get("K_WARM", "8"))
_SP = _os.environ.get("K_SPREAD", "88")
SPREADS = ((9, 7, 0), (8, 8, 0),
           (int(_SP[0]), int(_SP[1:]) if len(_SP) > 2 else int(_SP[1]), 0),
           (int(_SP[0]), int(_SP[1:]) if len(_SP) > 2 else int(_SP[1]), 0))
_GPJ = int(_os.environ.get("K_GPJ", "10"))
E2_CHUNKS = (("v", 0, 8), ("v", 8, _GPJ), ("g", _GPJ, 16)) if _GPJ > 8 else (
    ("v", 0, 8), ("g", 8, 16))
WG_GPS = True
DMA_SPLIT4 = True


def _build(with_c0: bool):
    nc = bacc.Bacc("TRN2", target_bir_lowering=False, debug=False)

    d_etf = nc.dram_tensor("etf", [128, DCH * 96], BF16, kind="ExternalInput")
    d_w1h = nc.dram_tensor("w1h", [128, DCH * H // 2], BF16, kind="ExternalInput")
    d_pf = nc.dram_tensor("pf", [128, 130], F32, kind="ExternalInput")
    d_pb = nc.dram_tensor("pb", [128, 449], BF16, kind="ExternalInput")
    if with_c0:
        d_c0 = nc.dram_tensor("c0t", [1, C], BF16, kind="ExternalInput")
    d_out = nc.dram_tensor("out_nd", [BS, C + 1], F32, kind="ExternalOutput")

    with tile.TileContext(nc) as tc:
        with (
            tc.tile_pool(name="consts", bufs=1) as consts,
            tc.tile_pool(name="big", bufs=1) as bigp,
            tc.tile_pool(name="work", bufs=2) as work,
            tc.tile_pool(name="ubuf", bufs=32) as ubuf,
            tc.tile_pool(name="ebuf", bufs=3) as ebuf,
            tc.tile_pool(name="e2buf", bufs=2) as e2buf,
            tc.tile_pool(name="pbig", bufs=3, space="PSUM") as pbig,
            tc.tile_pool(name="phead", bufs=1, space="PSUM") as phead,
            tc.tile_pool(name="psmall", bufs=1, space="PSUM") as psmall,
        ):
            # warmup tiles: memsets first so the junk-fed PE can spin as
            # early as possible (p-state ramp + HAM un-throttle)
            warm = consts.tile([1, 1], F32)
            nc.gpsimd.memset(warm, 0.0)
            wl = consts.tile([128, 128], BF16)
            nc.gpsimd.memset(wl, 0.0)
            wr = consts.tile([128, 512], BF16)
            nc.gpsimd.memset(wr, 0.0)
            ones_col = consts.tile([128, 1], BF16)
            nc.gpsimd.memset(ones_col, 1.0)
            if with_c0:
                ones_row_bf = consts.tile([1, 128], BF16)
                nc.gpsimd.memset(ones_row_bf, 1.0)
            for _ in range(N_WARM_MM):
                pw = pbig.tile([128, 1024], F32, tag="pb")
                nc.tensor.matmul(pw[:, 0:512], wl, wr, start=True, stop=True)

            # ---- input DMAs over three engine queues ----
            pf = consts.tile([128, 130], F32)
            pb = consts.tile([128, 449], BF16)
            b1h_sb, cc_sb = pf[:, 0:1], pf[:, 1:2]
            nfs_b, px_b = pf[:, 2:66], pf[:, 66:130]
            wh2_sb, am_sb, bm_sb = pb[:, 0:128], pb[:, 128:256], pb[:, 256:384]
            w2c_sb = pb[:, 384:449]
            if with_c0:
                c0_sb = consts.tile([1, C], BF16)
                nc.sync.dma_start(out=c0_sb, in_=d_c0.ap())

            etf = bigp.tile([128, DCH, 192], F8)
            w1h_sb = bigp.tile([128, DCH, H], F8)
            etf_view = d_etf.ap().bitcast(F8).rearrange("p (i c) -> p i c", i=DCH)
            w1h_view = d_w1h.ap().bitcast(F8).rearrange("p (i h) -> p i h", i=DCH)
            if DMA_SPLIT4:
                # fp8 payload ~725KB: sync etf(0:10) then pf | scalar
                # w1h(0:10) then pb | gpsimd (SWDGE) etf(10:16)+w1h(10:16).
                # pf/pb ride last: their consumers (x1 bias, head2/ab
                # weights, w2c, nfs/px) all run after head chunk 0 anyway.
                for a, b in ((0, 2), (2, 6), (6, 10)):
                    sl = slice(a, b)
                    nc.sync.dma_start(out=etf[:, sl, :], in_=etf_view[:, sl, :])
                nc.sync.dma_start(out=pf, in_=d_pf.ap())
                for a, b in ((0, 2), (2, 6), (6, 10)):
                    sl = slice(a, b)
                    nc.scalar.dma_start(out=w1h_sb[:, sl, :], in_=w1h_view[:, sl, :])
                nc.scalar.dma_start(out=pb, in_=d_pb.ap())
                sl = slice(10, 16)
                nc.gpsimd.dma_start(out=etf[:, sl, :], in_=etf_view[:, sl, :])
                sl = slice(10, 16)
                nc.gpsimd.dma_start(out=w1h_sb[:, sl, :], in_=w1h_view[:, sl, :])
                chunk_order = list(range(DCH))
            else:
                nc.sync.dma_start(out=pf, in_=d_pf.ap())
                nc.scalar.dma_start(out=pb, in_=d_pb.ap())
                for a, b in ((0, 2), (2, 4), (4, 8), (8, 16)):
                    sl = slice(a, b)
                    nc.sync.dma_start(out=etf[:, sl, :], in_=etf_view[:, sl, :])
                    nc.scalar.dma_start(out=w1h_sb[:, sl, :], in_=w1h_view[:, sl, :])
                chunk_order = list(range(DCH))
            warm2 = consts.tile([1, 1], F32)
            nc.scalar.activation(warm2, warm, ACTF.Exp)

            # ---- head layer 1: X1 = relu(W1h.T @ [embT | all_fT] + b1h) ----
            xt = phead.tile([128, 192], F32, tag="ph")
            for k, i in enumerate(chunk_order):
                nc.tensor.matmul(
                    xt, w1h_sb[:, i, :], etf[:, i, :], start=(k == 0),
                    stop=(k == DCH - 1), skip_group_check=True,
                )
            x1 = work.tile([128, 192], BF16)
            nc.scalar.activation(
                x1[:, 0:128], xt[:, 0:128], ACTF.Relu, bias=b1h_sb,
                scale=1.0 / 256.0,
            )
            nc.scalar.activation(
                x1[:, 128:192], xt[:, 128:192], ACTF.Relu, bias=b1h_sb,
                scale=1.0 / 16.0,
            )
            # head layer 2 (head_b2 folded into cc)
            x2p = phead.tile([128, 192], F32, tag="ph")
            nc.tensor.matmul(x2p, wh2_sb, x1, skip_group_check=True)
            x2 = work.tile([128, 192], BF16)
            nc.scalar.copy(x2, x2p)
            hz2 = x2[:, 0:128]
            hf2 = x2[:, 128:192]
            # mixer layer 1 collapses: alpha[h, b] (+cc), beta[h, n]
            abp = phead.tile([128, 192], F32, tag="ph")
            nc.tensor.matmul(abp[:, 0:64], am_sb, hf2, skip_group_check=True)
            nc.tensor.matmul(abp[:, 64:192], bm_sb, hz2, skip_group_check=True)
            # z_x scores gate only ez / accum(0): first half fills the PE
            # while alpha/betaT resolve, second half goes after logits(0)
            slp = psmall.tile([NLOC, BS], F32, tag="ps")
            for k, i in enumerate(chunk_order[:8]):
                nc.tensor.matmul(
                    slp, etf[:, i, 0:128], etf[:, i, 128:192], start=(k == 0),
                    stop=False, skip_group_check=True,
                )
            alpha = work.tile([128, 64], F32)
            nc.scalar.activation(alpha, abp[:, 0:64], ACTF.Identity, bias=cc_sb)
            betaT = work.tile([128, 128], BF16)
            nc.vector.tensor_copy(betaT, abp[:, 64:192])

            e_tiles = [None] * NG
            e2_tiles = [None] * NG
            u_tiles = [[None] * GRP for _ in range(NG)]

            def emit_us(g, which="vsg"):
                # u producers for group g, spread (dve, scalar, gpsimd);
                # `which` selects the engine subset to emit now so each
                # engine gets its u block at the right point of its queue
                nv, ns, ng_ = SPREADS[g]
                for jg in range(GRP):
                    b = GRP * g + jg
                    a_col = alpha[:, b : b + 1]
                    if jg < nv:
                        if "v" not in which:
                            continue
                        u = ubuf.tile([128, 128], BF16, tag="u", name="u")
                        nc.vector.tensor_scalar(
                            u, betaT, a_col, 0.0, op0=ALU.add, op1=ALU.max
                        )
                    elif jg < nv + ns:
                        if "s" not in which:
                            continue
                        u = ubuf.tile([128, 128], BF16, tag="u", name="u")
                        nc.scalar.activation(u, betaT, ACTF.Relu, bias=a_col)
                    else:
                        if "g" not in which:
                            continue
                        u = ubuf.tile([128, 128], BF16, tag="u", name="u")
                        nc.gpsimd.tensor_scalar(
                            u, betaT, a_col, 0.0, op0=ALU.add, op1=ALU.max
                        )
                    u_tiles[g][jg] = u

            def emit_front(g):
                # logits matmuls + per-half exp (u's made a group ahead)
                e_g = ebuf.tile([128, GRP, CP], BF16, tag="e")
                e_tiles[g] = e_g
                for h in range(2):
                    pg = pbig.tile([128, 1024], F32, tag="pb")
                    for j in range(HALF):
                        jg = HALF * h + j
                        u = u_tiles[g][jg]
                        off = 512 * (j // 4) + C * (j % 4)
                        sl = pg[:, off : off + C]
                        if with_c0:
                            nc.tensor.matmul(
                                sl, ones_row_bf, c0_sb, start=True, stop=False,
                                skip_group_check=True,
                            )
                            nc.tensor.matmul(
                                sl, u, w2c_sb, start=False, stop=True,
                                skip_group_check=True,
                            )
                        else:
                            nc.tensor.matmul(
                                sl, u, w2c_sb, start=True, stop=True,
                                skip_group_check=True,
                            )
                    pg_v = pg.rearrange("p (u x) -> p u x", u=2)[:, :, 0 : 4 * C]
                    pg_v = pg_v.rearrange("p u (j c) -> p u j c", c=C)
                    eh = e_g[:, HALF * h : HALF * (h + 1), 0:C]
                    nc.scalar.activation(
                        eh.rearrange("p (u j) c -> p u j c", u=2), pg_v, ACTF.Exp
                    )

            def emit_back(g):
                # per-half softmax denominators + p_x scale on DVE; each e2
                # chunk is emitted as soon as its half's wg exists so the
                # accum matmuls can start early
                e_g = e_tiles[g]
                wg = work.tile([128, GRP], BF16, tag="wg", name="wg")

                def e2_chunk(eng, j0, j1):
                    veng = nc.vector if eng == "v" else nc.gpsimd
                    self_sl = slice(j0, j1)
                    wv = (
                        wg[:, self_sl]
                        .unsqueeze(2)
                        .broadcast_to([128, j1 - j0, C])
                    )
                    veng.tensor_tensor(
                        e2_tiles[g][:, self_sl, 0:C],
                        e_g[:, self_sl, 0:C],
                        wv,
                        op=ALU.mult,
                    )

                for h in range(2):
                    hs = slice(HALF * h, HALF * (h + 1))
                    bs0 = GRP * g + HALF * h
                    sg = work.tile([128, HALF], F32, tag=f"sg{h}", name="sg")
                    nc.vector.reduce_sum(sg, e_g[:, hs, 0:C], axis=AX.X)
                    rg = work.tile([128, HALF], F32, tag=f"rg{h}", name="rg")
                    nc.vector.reciprocal_approx_fast(rg, sg)
                    weng = nc.gpsimd if WG_GPS else nc.vector
                    weng.tensor_tensor(
                        wg[:, hs], rg, px_b[:, bs0 : bs0 + HALF], op=ALU.mult
                    )
                    for eng, j0, j1 in E2_CHUNKS:
                        if HALF * h <= j0 < HALF * (h + 1):
                            e2_chunk(eng, j0, j1)
                for eng, j0, j1 in E2_CHUNKS:
                    if not (0 <= j0 < GRP) or not any(
                        HALF * h <= j0 < HALF * (h + 1) for h in range(2)
                    ):
                        e2_chunk(eng, j0, j1)

            def emit_accum(g):
                e2_g = e2_tiles[g]
                for j in range(GRP):
                    b = GRP * g + j
                    nc.tensor.matmul(
                        onp[:, 0:C], ez, e2_g[:, j, 0:C],
                        start=(b == 0), stop=(b == BS - 1),
                        skip_group_check=True,
                    )

            # pipeline: u's one group ahead; back(g) = denom/e2 right after
            # front(g+1); accum(g) follows immediately (same period)
            emit_us(0)
            emit_us(1)
            emit_front(0)
            # remaining score chunks ride the PE behind logits(0); then the
            # ez row and z_x denominator column resolve well before accum(0)
            for k, i in enumerate(chunk_order[8:]):
                nc.tensor.matmul(
                    slp, etf[:, i, 0:128], etf[:, i, 128:192], start=False,
                    stop=(k == DCH - 9), skip_group_check=True,
                )
            t3 = work.tile([NLOC, BS], F32)
            nc.vector.tensor_tensor(t3, slp, nfs_b, op=ALU.mult)
            ez = work.tile([NLOC, BS], BF16)
            nc.scalar.activation(ez, t3, ACTF.Exp)
            onp = psmall.tile([BS, C + 1], F32, tag="ps")
            for g in range(1, NG):
                e2_tiles[g - 1] = e2buf.tile([128, GRP, CP], BF16, tag="e2", name="e2g")
                emit_front(g)
                if g + 1 < NG:
                    emit_us(g + 1, "s")
                emit_back(g - 1)
                if g == 1:
                    # z_x denominator column: needs only ez; placed here so
                    # the in-order PE queue never stalls on it
                    nc.tensor.matmul(onp[:, C : C + 1], ez, ones_col)
                emit_accum(g - 1)
                if g + 1 < NG:
                    emit_us(g + 1, "v")
                    emit_us(g + 1, "g")
            e2_tiles[NG - 1] = e2buf.tile([128, GRP, CP], BF16, tag="e2", name="e2g")
            emit_back(NG - 1)
            emit_accum(NG - 1)

            # ---- ship the partial result ----
            on_s = work.tile([BS, C + 1], F32)
            nc.scalar.copy(on_s, onp)
            nc.sync.dma_start(out=d_out.ap(), in_=on_s)

    nc.compile()
    return nc


_CACHE: dict = {}
LAST_RESULTS = None  # BassKernelResults of the most recent run (for profiling)


def _get_nc(with_c0: bool):
    if with_c0 not in _CACHE:
        _CACHE[with_c0] = _build(with_c0)
    return _CACHE[with_c0]


def kernel(
    all_f, embedding, all_y,
    head_w1, head_b1, head_g, head_beta, head_rm, head_rv, head_w2, head_b2,
    mix_w1, mix_b1, mix_g, mix_beta, mix_rm, mix_rv, mix_w2, mix_b2,
    clf_w, clf_b,
):
    f64 = np.float64
    bf16 = ml_dtypes.bfloat16
    sh = head_g.astype(f64) / np.sqrt(head_rv.astype(f64) + BN_EPS)
    th = head_beta.astype(f64) - head_rm.astype(f64) * sh
    w1h = head_w1.astype(f64) * sh[None, :]
    b1h = (head_b1.astype(f64) * sh + th).astype(np.float32)[:, None]
    sm = mix_g.astype(f64) / np.sqrt(mix_rv.astype(f64) + BN_EPS)
    tm = mix_beta.astype(f64) - mix_rm.astype(f64) * sm
    am = mix_w1[:H].astype(f64) * sm[None, :]
    bm = mix_w1[H:].astype(f64) * sm[None, :]
    cm = mix_b1.astype(f64) * sm + tm
    ca = (head_b2.astype(f64) @ am + cm).astype(np.float32)[:, None]
    cb = (head_b2.astype(f64) @ bm).astype(np.float32)[:, None]
    w2c = (mix_w2.astype(f64) @ clf_w.astype(f64)).astype(bf16)
    c0 = (mix_b2.astype(f64) @ clf_w.astype(f64) + clf_b.astype(f64)).astype(
        np.float32
    )
    with_c0 = bool(np.any(c0 != 0.0))

    af = np.ascontiguousarray(all_f, dtype=np.float32)
    emb = np.ascontiguousarray(embedding, dtype=np.float32)
    # input-side host prep: row norms folded into the bf16 payloads, p_x
    nf = 1.0 / np.sqrt((af.astype(f64) ** 2).sum(axis=1))           # [bs]
    nfs = (nf / (T * np.sqrt(N)) / 16.0).astype(np.float32)
    gscore = ((af @ af.T).astype(f64) * nf[:, None] * nf[None, :]).sum(axis=1)
    gscore = gscore / (T * np.sqrt(BS))
    pe = np.exp(gscore - gscore.max())
    px = pe / pe.sum()                                              # [bs]
    f8 = ml_dtypes.float8_e4m3
    aft = np.ascontiguousarray(af.T).astype(f8)                     # [D, bs]

    pack_bf = np.empty((128, 449), dtype=bf16)
    pack_bf[:, 0:128] = np.ascontiguousarray(head_w2).astype(bf16)
    pack_bf[:, 128:256] = am.astype(bf16)
    pack_bf[:, 256:384] = bm.astype(bf16)
    pack_bf[:, 384:449] = w2c
    pack_f = np.empty((128, 130), dtype=np.float32)
    pack_f[:, 0:1] = b1h
    pack_f[:, 1:2] = ca + cb
    pack_f[:, 2:66] = np.broadcast_to(nfs[None, :], (128, BS))
    pack_f[:, 66:130] = np.broadcast_to(px[None, :].astype(np.float32), (128, BS))
    # w1h repack: [128, DCH*H], row p holds chunks i (w1h[i*128+p, :])
    w1h_p = np.ascontiguousarray(
        (w1h * 16.0).reshape(DCH, 128, H).transpose(1, 0, 2).reshape(128, DCH * H)
    ).astype(f8)
    base = {"w1h": w1h_p.view(bf16), "pb": pack_bf, "pf": pack_f}
    if with_c0:
        base["c0t"] = c0[None, :].astype(bf16)

    ne = 1.0 / np.sqrt((emb.astype(f64) ** 2).sum(axis=1))          # [N]
    embn = (emb * ne[:, None].astype(np.float32)).astype(np.float64)

    in_maps = []
    for i in range(NCORES):
        shard = embn[i * NLOC : (i + 1) * NLOC]                     # [128, D]
        # etf packed [128, DCH*192]: row p, chunk i = [embT | afT] rows i*128+p
        etf3 = np.empty((128, DCH, 192), dtype=f8)
        shard_t = (shard.T * 16.0).reshape(DCH, 128, NLOC)          # [DCH,128,128]
        etf3[:, :, 0:128] = shard_t.transpose(1, 0, 2).astype(f8)
        etf3[:, :, 128:192] = aft.reshape(DCH, 128, BS).transpose(1, 0, 2)
        in_maps.append(dict(base, etf=etf3.reshape(128, DCH * 192).view(bf16)))

    nc = _get_nc(with_c0)
    try:
        res = bass_utils.run_bass_kernel_spmd(
            nc, in_maps, core_ids=list(range(NCORES))
        )
    except Exception:
        # fresh NEFFs occasionally hit a transient NRT exec fault on their
        # first dispatch; one retry has always succeeded
        res = bass_utils.run_bass_kernel_spmd(
            nc, in_maps, core_ids=list(range(NCORES))
        )
    global LAST_RESULTS
    LAST_RESULTS = res
    parts = np.stack([r["out_nd"] for r in res.results], axis=0)  # [8, 64, 66]
    tot = parts.sum(axis=0)
    return (tot[:, :C] / tot[:, C : C + 1]).astype(np.float32)


# revision 22
# speedup vs baseline: 1.0239x; 1.0239x over previous
"""Trainium2 Bass kernel for the CCM retrieval problem.

Reference computation (shapes: bs=64, N=1024, D=2048, H=128, C=65):
    z_x   = softmax(cos(all_f, emb)/T/sqrt(N))            [bs, N]
    hf    = head(all_f); hz = head(emb)                   [bs, H], [N, H]
    h1    = relu(BN(hf[b] @ A + b1 + hz[n] @ B))          [bs, N, H]
    y_zx  = softmax((h1 @ mix_w2 + mix_b2) @ clf_w + clf_b)  [bs, N, C]
    p_x   = softmax(sum_n cos(all_f, all_f)/T/sqrt(bs))   [bs]
    out   = z_x @ einsum('b,bnc->nc', p_x, y_zx)          [bs, C]

Device strategy: shard the queue axis N across 8 cores (128 rows each); bs
stays replicated so sum_x is core-local.  Host folds the BN affines into
weights, pre-multiplies mix_w2 @ clf_w (W2C) so the [bs,N,D] intermediate
never exists, pre-normalizes the embedding rows, and precomputes p_x.

Per core: input DMA is spread over four engine queues (sync/vector carry
etf halves, scalar/gpsimd carry w1h halves) so the ~1.3MB payload lands in
half the time; head-layer matmuls are emitted in chunk-arrival order.  The
mixer's first layer collapses to alpha[h,b] (64 cols) and beta[h,n] (128
cols); per b a fused relu(beta + alpha[:,b]) tensor_scalar produces the
logits stationary, spread across DVE (4x mode) / ScalarE / GpSimd one group
ahead of the PE.  exp on ScalarE; the softmax row-sums, reciprocal and
p_x-scale run per 8-b half on DVE so the e2 weighting (split DVE/GpSimd)
and the PSUM-accumulated b-sum start as early as possible.  Each core
returns [64, 66]: columns 0:65 are exp(z-score) @ sum_x partial numerators,
column 65 the z_x softmax denominator partial; the host sums partials over
cores and divides.
"""

import numpy as np
import ml_dtypes

import concourse.bass as bass
import concourse.tile as tile
from concourse import bacc, mybir
from concourse import bass_utils

F32 = mybir.dt.float32
BF16 = mybir.dt.bfloat16
F8 = mybir.dt.float8e4
AX = mybir.AxisListType
ALU = mybir.AluOpType
ACTF = mybir.ActivationFunctionType

T = 0.07
BN_EPS = 1e-5
BS, D, N, H, C = 64, 2048, 1024, 128, 65
NCORES = 8
NLOC = N // NCORES          # 128 queue rows per core
DCH = D // 128              # 16 contraction chunks
CP = 66                     # padded C stride (even -> 4B-aligned bf16 rows)
GRP = 16                    # b's per group (S/e2/accum granularity)
NG = BS // GRP              # 4 groups
HALF = 8                    # b's per 2-bank psum tile (4 per bank)

# tuning knobs --------------------------------------------------------------
# N_WARM_MM: junk matmuls before the head to ramp the PE p-state / HAM.
# SPREADS[g]: (n_dve, n_scalar, n_gpsimd) u-producers for group g; u's for
#   group g are emitted one group ahead of their logits matmuls.
# E2_CHUNKS: (engine, j0, j1) pieces of the e2 = e * w multiply.
import os as _os
N_WARM_MM = int(_os.environ.get("K_WARM", "8"))
_SP = _os.environ.get("K_SPREAD", "88")
SPREADS = ((9, 7, 0), (8, 8, 0),
           (int(_SP[0]), int(_SP[1:]) if len(_SP) > 2 else int(_SP[1]), 0),
           (int(_SP[0]), int(_SP[1:]) if len(_SP) > 2 else int(_SP[1]), 0))
_GPJ = int(_os.environ.get("K_GPJ", "10"))
E2_CHUNKS = (("v", 0, 8), ("v", 8, _GPJ), ("g", _GPJ, 16)) if _GPJ > 8 else (
    ("v", 0, 8), ("g", 8, 16))
WG_GPS = True
DMA_SPLIT4 = True


def _build(with_c0: bool):
    nc = bacc.Bacc("TRN2", target_bir_lowering=False, debug=False)

    d_etf = nc.dram_tensor("etf", [128, DCH * 96], BF16, kind="ExternalInput")
    d_w1h = nc.dram_tensor("w1h", [128, DCH * H // 2], BF16, kind="ExternalInput")
    d_pf = nc.dram_tensor("pf", [128, 130], F32, kind="ExternalInput")
    d_pb = nc.dram_tensor("pb", [128, 449], BF16, kind="ExternalInput")
    if with_c0:
        d_c0 = nc.dram_tensor("c0t", [1, C], BF16, kind="ExternalInput")
    d_out = nc.dram_tensor("out_nd", [BS, C + 1], F32, kind="ExternalOutput")

    with tile.TileContext(nc) as tc:
        with (
            tc.tile_pool(name="consts", bufs=1) as consts,
            tc.tile_pool(name="big", bufs=1) as bigp,
            tc.tile_pool(name="work", bufs=2) as work,
            tc.tile_pool(name="ubuf", bufs=32) as ubuf,
            tc.tile_pool(name="ebuf", bufs=3) as ebuf,
            tc.tile_pool(name="e2buf", bufs=2) as e2buf,
            tc.tile_pool(name="pbig", bufs=3, space="PSUM") as pbig,
            tc.tile_pool(name="phead", bufs=1, space="PSUM") as phead,
            tc.tile_pool(name="psmall", bufs=1, space="PSUM") as psmall,
        ):
            # warmup tiles: memsets first so the junk-fed PE can spin as
            # early as possible (p-state ramp + HAM un-throttle)
            warm = consts.tile([1, 1], F32)
            nc.gpsimd.memset(warm, 0.0)
            wl = consts.tile([128, 128], BF16)
            nc.gpsimd.memset(wl, 0.0)
            wr = consts.tile([128, 512], BF16)
            nc.gpsimd.memset(wr, 0.0)
            ones_col = consts.tile([128, 1], BF16)
            nc.gpsimd.memset(ones_col, 1.0)
            if with_c0:
                ones_row_bf = consts.tile([1, 128], BF16)
                nc.gpsimd.memset(ones_row_bf, 1.0)
            for _ in range(N_WARM_MM):
                pw = pbig.tile([128, 1024], F32, tag="pb")
                nc.tensor.matmul(pw[:, 0:512], wl, wr, start=True, stop=True)

            # ---- input DMAs over three engine queues ----
            pf = consts.tile([128, 130], F32)
            pb = consts.tile([128, 449], BF16)
            b1h_sb, cc_sb = pf[:, 0:1], pf[:, 1:2]
            nfs_b, px_b = pf[:, 2:66], pf[:, 66:130]
            wh2_sb, am_sb, bm_sb = pb[:, 0:128], pb[:, 128:256], pb[:, 256:384]
            w2c_sb = pb[:, 384:449]
            if with_c0:
                c0_sb = consts.tile([1, C], BF16)
                nc.sync.dma_start(out=c0_sb, in_=d_c0.ap())

            etf = bigp.tile([128, DCH, 192], F8)
            w1h_sb = bigp.tile([128, DCH, H], F8)
            etf_view = d_etf.ap().bitcast(F8).rearrange("p (i c) -> p i c", i=DCH)
            w1h_view = d_w1h.ap().bitcast(F8).rearrange("p (i h) -> p i h", i=DCH)
            if DMA_SPLIT4:
                # fp8 payload ~725KB: sync etf(0:10) then pf | scalar
                # w1h(0:10) then pb | gpsimd (SWDGE) etf(10:16)+w1h(10:16).
                # pf/pb ride last: their consumers (x1 bias, head2/ab
                # weights, w2c, nfs/px) all run after head chunk 0 anyway.
                for a, b in ((0, 2), (2, 6), (6, 10)):
                    sl = slice(a, b)
                    nc.sync.dma_start(out=etf[:, sl, :], in_=etf_view[:, sl, :])
                nc.sync.dma_start(out=pf, in_=d_pf.ap())
                for a, b in ((0, 2), (2, 6), (6, 10)):
                    sl = slice(a, b)
                    nc.scalar.dma_start(out=w1h_sb[:, sl, :], in_=w1h_view[:, sl, :])
                nc.scalar.dma_start(out=pb, in_=d_pb.ap())
                sl = slice(10, 16)
                nc.gpsimd.dma_start(out=etf[:, sl, :], in_=etf_view[:, sl, :])
                sl = slice(10, 16)
                nc.gpsimd.dma_start(out=w1h_sb[:, sl, :], in_=w1h_view[:, sl, :])
                chunk_order = list(range(DCH))
            else:
                nc.sync.dma_start(out=pf, in_=d_pf.ap())
                nc.scalar.dma_start(out=pb, in_=d_pb.ap())
                for a, b in ((0, 2), (2, 4), (4, 8), (8, 16)):
                    sl = slice(a, b)
                    nc.sync.dma_start(out=etf[:, sl, :], in_=etf_view[:, sl, :])
                    nc.scalar.dma_start(out=w1h_sb[:, sl, :], in_=w1h_view[:, sl, :])
                chunk_order = list(range(DCH))
            warm2 = consts.tile([1, 1], F32)
            nc.scalar.activation(warm2, warm, ACTF.Exp)

            # ---- head layer 1: X1 = relu(W1h.T @ [embT | all_fT] + b1h) ----
            xt = phead.tile([128, 192], F32, tag="ph")
            for k, i in enumerate(chunk_order):
                nc.tensor.matmul(
                    xt, w1h_sb[:, i, :], etf[:, i, :], start=(k == 0),
                    stop=(k == DCH - 1), skip_group_check=True,
                )
            x1 = work.tile([128, 192], BF16)
            nc.scalar.activation(
                x1[:, 0:128], xt[:, 0:128], ACTF.Relu, bias=b1h_sb,
                scale=1.0 / 256.0,
            )
            nc.scalar.activation(
                x1[:, 128:192], xt[:, 128:192], ACTF.Relu, bias=b1h_sb,
                scale=1.0 / 16.0,
            )
            # head layer 2 (head_b2 folded into cc)
            x2p = phead.tile([128, 192], F32, tag="ph")
            nc.tensor.matmul(x2p, wh2_sb, x1, skip_group_check=True)
            x2 = work.tile([128, 192], BF16)
            nc.scalar.copy(x2, x2p)
            hz2 = x2[:, 0:128]
            hf2 = x2[:, 128:192]
            # mixer layer 1 collapses: alpha[h, b] (+cc), beta[h, n]
            abp = phead.tile([128, 192], F32, tag="ph")
            nc.tensor.matmul(abp[:, 0:64], am_sb, hf2, skip_group_check=True)
            nc.tensor.matmul(abp[:, 64:192], bm_sb, hz2, skip_group_check=True)
            # z_x scores gate only ez / accum(0): first half fills the PE
            # while alpha/betaT resolve, second half goes after logits(0)
            slp = psmall.tile([NLOC, BS], F32, tag="ps")
            for k, i in enumerate(chunk_order[:8]):
                nc.tensor.matmul(
                    slp, etf[:, i, 0:128], etf[:, i, 128:192], start=(k == 0),
                    stop=False, skip_group_check=True,
                )
            alpha = work.tile([128, 64], F32)
            nc.scalar.activation(alpha, abp[:, 0:64], ACTF.Identity, bias=cc_sb)
            betaT = work.tile([128, 128], BF16)
            nc.vector.tensor_copy(betaT, abp[:, 64:192])

            e_tiles = [None] * NG
            e2_tiles = [None] * NG
            u_tiles = [[None] * GRP for _ in range(NG)]

            def emit_us(g, which="vsg", s_lo=0, s_hi=GRP):
                # u producers for group g, spread (dve, scalar, gpsimd);
                # `which` selects the engine subset to emit now (and for
                # scalar, the sub-block [s_lo, s_hi) of its u's) so each
                # engine gets its u work at the right point of its queue
                nv, ns, ng_ = SPREADS[g]
                s_seen = 0
                for jg in range(GRP):
                    b = GRP * g + jg
                    a_col = alpha[:, b : b + 1]
                    if jg < nv:
                        if "v" not in which:
                            continue
                        u = ubuf.tile([128, 128], BF16, tag="u", name="u")
                        nc.vector.tensor_scalar(
                            u, betaT, a_col, 0.0, op0=ALU.add, op1=ALU.max
                        )
                    elif jg < nv + ns:
                        si = s_seen
                        s_seen += 1
                        if "s" not in which or not (s_lo <= si < s_hi):
                            continue
                        u = ubuf.tile([128, 128], BF16, tag="u", name="u")
                        nc.scalar.activation(u, betaT, ACTF.Relu, bias=a_col)
                    else:
                        if "g" not in which:
                            continue
                        u = ubuf.tile([128, 128], BF16, tag="u", name="u")
                        nc.gpsimd.tensor_scalar(
                            u, betaT, a_col, 0.0, op0=ALU.add, op1=ALU.max
                        )
                    u_tiles[g][jg] = u

            def emit_front(g, halves=(0, 1)):
                # logits matmuls + per-half exp (u's made a group ahead)
                if 0 in halves:
                    e_g = ebuf.tile([128, GRP, CP], BF16, tag="e")
                    e_tiles[g] = e_g
                e_g = e_tiles[g]
                for h in halves:
                    pg = pbig.tile([128, 1024], F32, tag="pb")
                    for j in range(HALF):
                        jg = HALF * h + j
                        u = u_tiles[g][jg]
                        off = 512 * (j // 4) + C * (j % 4)
                        sl = pg[:, off : off + C]
                        if with_c0:
                            nc.tensor.matmul(
                                sl, ones_row_bf, c0_sb, start=True, stop=False,
                                skip_group_check=True,
                            )
                            nc.tensor.matmul(
                                sl, u, w2c_sb, start=False, stop=True,
                                skip_group_check=True,
                            )
                        else:
                            nc.tensor.matmul(
                                sl, u, w2c_sb, start=True, stop=True,
                                skip_group_check=True,
                            )
                    pg_v = pg.rearrange("p (u x) -> p u x", u=2)[:, :, 0 : 4 * C]
                    pg_v = pg_v.rearrange("p u (j c) -> p u j c", c=C)
                    eh = e_g[:, HALF * h : HALF * (h + 1), 0:C]
                    nc.scalar.activation(
                        eh.rearrange("p (u j) c -> p u j c", u=2), pg_v, ACTF.Exp
                    )

            def emit_back(g):
                # per-half softmax denominators + p_x scale on DVE; each e2
                # chunk is emitted as soon as its half's wg exists so the
                # accum matmuls can start early
                e_g = e_tiles[g]
                wg = work.tile([128, GRP], BF16, tag="wg", name="wg")

                def e2_chunk(eng, j0, j1):
                    veng = nc.vector if eng == "v" else nc.gpsimd
                    self_sl = slice(j0, j1)
                    wv = (
                        wg[:, self_sl]
                        .unsqueeze(2)
                        .broadcast_to([128, j1 - j0, C])
                    )
                    veng.tensor_tensor(
                        e2_tiles[g][:, self_sl, 0:C],
                        e_g[:, self_sl, 0:C],
                        wv,
                        op=ALU.mult,
                    )

                for h in range(2):
                    hs = slice(HALF * h, HALF * (h + 1))
                    bs0 = GRP * g + HALF * h
                    sg = work.tile([128, HALF], F32, tag=f"sg{h}", name="sg")
                    nc.vector.reduce_sum(sg, e_g[:, hs, 0:C], axis=AX.X)
                    rg = work.tile([128, HALF], F32, tag=f"rg{h}", name="rg")
                    nc.vector.reciprocal_approx_fast(rg, sg)
                    weng = nc.gpsimd if WG_GPS else nc.vector
                    weng.tensor_tensor(
                        wg[:, hs], rg, px_b[:, bs0 : bs0 + HALF], op=ALU.mult
                    )
                    for eng, j0, j1 in E2_CHUNKS:
                        if HALF * h <= j0 < HALF * (h + 1):
                            e2_chunk(eng, j0, j1)
                for eng, j0, j1 in E2_CHUNKS:
                    if not (0 <= j0 < GRP) or not any(
                        HALF * h <= j0 < HALF * (h + 1) for h in range(2)
                    ):
                        e2_chunk(eng, j0, j1)

            def emit_accum(g):
                e2_g = e2_tiles[g]
                for j in range(GRP):
                    b = GRP * g + j
                    nc.tensor.matmul(
                        onp[:, 0:C], ez, e2_g[:, j, 0:C],
                        start=(b == 0), stop=(b == BS - 1),
                        skip_group_check=True,
                    )

            # pipeline: u's one group ahead; back(g) = denom/e2 right after
            # front(g+1); accum(g) follows immediately (same period)
            emit_us(0)
            emit_us(1)
            emit_front(0)
            # remaining score chunks ride the PE behind logits(0); then the
            # ez row and z_x denominator column resolve well before accum(0)
            for k, i in enumerate(chunk_order[8:]):
                nc.tensor.matmul(
                    slp, etf[:, i, 0:128], etf[:, i, 128:192], start=False,
                    stop=(k == DCH - 9), skip_group_check=True,
                )
            t3 = work.tile([NLOC, BS], F32)
            nc.vector.tensor_tensor(t3, slp, nfs_b, op=ALU.mult)
            ez = work.tile([NLOC, BS], BF16)
            nc.scalar.activation(ez, t3, ACTF.Exp)
            onp = psmall.tile([BS, C + 1], F32, tag="ps")
            for g in range(1, NG):
                e2_tiles[g - 1] = e2buf.tile([128, GRP, CP], BF16, tag="e2", name="e2g")
                if g + 1 < NG:
                    emit_us(g + 1, "s", 0, 3)
                    emit_front(g, (0,))
                    emit_us(g + 1, "s", 3, 5)
                    emit_front(g, (1,))
                    emit_us(g + 1, "s", 5, GRP)
                else:
                    emit_front(g)
                emit_back(g - 1)
                if g == 1:
                    # z_x denominator column: needs only ez; placed here so
                    # the in-order PE queue never stalls on it
                    nc.tensor.matmul(onp[:, C : C + 1], ez, ones_col)
                emit_accum(g - 1)
                if g + 1 < NG:
                    emit_us(g + 1, "v")
                    emit_us(g + 1, "g")
            e2_tiles[NG - 1] = e2buf.tile([128, GRP, CP], BF16, tag="e2", name="e2g")
            emit_back(NG - 1)
            emit_accum(NG - 1)

            # ---- ship the partial result ----
            on_s = work.tile([BS, C + 1], F32)
            nc.scalar.copy(on_s, onp)
            nc.sync.dma_start(out=d_out.ap(), in_=on_s)

    nc.compile()
    return nc


_CACHE: dict = {}
LAST_RESULTS = None  # BassKernelResults of the most recent run (for profiling)


def _get_nc(with_c0: bool):
    if with_c0 not in _CACHE:
        _CACHE[with_c0] = _build(with_c0)
    return _CACHE[with_c0]


def kernel(
    all_f, embedding, all_y,
    head_w1, head_b1, head_g, head_beta, head_rm, head_rv, head_w2, head_b2,
    mix_w1, mix_b1, mix_g, mix_beta, mix_rm, mix_rv, mix_w2, mix_b2,
    clf_w, clf_b,
):
    f64 = np.float64
    bf16 = ml_dtypes.bfloat16
    sh = head_g.astype(f64) / np.sqrt(head_rv.astype(f64) + BN_EPS)
    th = head_beta.astype(f64) - head_rm.astype(f64) * sh
    w1h = head_w1.astype(f64) * sh[None, :]
    b1h = (head_b1.astype(f64) * sh + th).astype(np.float32)[:, None]
    sm = mix_g.astype(f64) / np.sqrt(mix_rv.astype(f64) + BN_EPS)
    tm = mix_beta.astype(f64) - mix_rm.astype(f64) * sm
    am = mix_w1[:H].astype(f64) * sm[None, :]
    bm = mix_w1[H:].astype(f64) * sm[None, :]
    cm = mix_b1.astype(f64) * sm + tm
    ca = (head_b2.astype(f64) @ am + cm).astype(np.float32)[:, None]
    cb = (head_b2.astype(f64) @ bm).astype(np.float32)[:, None]
    w2c = (mix_w2.astype(f64) @ clf_w.astype(f64)).astype(bf16)
    c0 = (mix_b2.astype(f64) @ clf_w.astype(f64) + clf_b.astype(f64)).astype(
        np.float32
    )
    with_c0 = bool(np.any(c0 != 0.0))

    af = np.ascontiguousarray(all_f, dtype=np.float32)
    emb = np.ascontiguousarray(embedding, dtype=np.float32)
    # input-side host prep: row norms folded into the bf16 payloads, p_x
    nf = 1.0 / np.sqrt((af.astype(f64) ** 2).sum(axis=1))           # [bs]
    nfs = (nf / (T * np.sqrt(N)) / 16.0).astype(np.float32)
    gscore = ((af @ af.T).astype(f64) * nf[:, None] * nf[None, :]).sum(axis=1)
    gscore = gscore / (T * np.sqrt(BS))
    pe = np.exp(gscore - gscore.max())
    px = pe / pe.sum()                                              # [bs]
    f8 = ml_dtypes.float8_e4m3
    aft = np.ascontiguousarray(af.T).astype(f8)                     # [D, bs]

    pack_bf = np.empty((128, 449), dtype=bf16)
    pack_bf[:, 0:128] = np.ascontiguousarray(head_w2).astype(bf16)
    pack_bf[:, 128:256] = am.astype(bf16)
    pack_bf[:, 256:384] = bm.astype(bf16)
    pack_bf[:, 384:449] = w2c
    pack_f = np.empty((128, 130), dtype=np.float32)
    pack_f[:, 0:1] = b1h
    pack_f[:, 1:2] = ca + cb
    pack_f[:, 2:66] = np.broadcast_to(nfs[None, :], (128, BS))
    pack_f[:, 66:130] = np.broadcast_to(px[None, :].astype(np.float32), (128, BS))
    # w1h repack: [128, DCH*H], row p holds chunks i (w1h[i*128+p, :])
    w1h_p = np.ascontiguousarray(
        (w1h * 16.0).reshape(DCH, 128, H).transpose(1, 0, 2).reshape(128, DCH * H)
    ).astype(f8)
    base = {"w1h": w1h_p.view(bf16), "pb": pack_bf, "pf": pack_f}
    if with_c0:
        base["c0t"] = c0[None, :].astype(bf16)

    ne = 1.0 / np.sqrt((emb.astype(f64) ** 2).sum(axis=1))          # [N]
    embn = (emb * ne[:, None].astype(np.float32)).astype(np.float64)

    in_maps = []
    for i in range(NCORES):
        shard = embn[i * NLOC : (i + 1) * NLOC]                     # [128, D]
        # etf packed [128, DCH*192]: row p, chunk i = [embT | afT] rows i*128+p
        etf3 = np.empty((128, DCH, 192), dtype=f8)
        shard_t = (shard.T * 16.0).reshape(DCH, 128, NLOC)          # [DCH,128,128]
        etf3[:, :, 0:128] = shard_t.transpose(1, 0, 2).astype(f8)
        etf3[:, :, 128:192] = aft.reshape(DCH, 128, BS).transpose(1, 0, 2)
        in_maps.append(dict(base, etf=etf3.reshape(128, DCH * 192).view(bf16)))

    nc = _get_nc(with_c0)
    try:
        res = bass_utils.run_bass_kernel_spmd(
            nc, in_maps, core_ids=list(range(NCORES))
        )
    except Exception:
        # fresh NEFFs occasionally hit a transient NRT exec fault on their
        # first dispatch; one retry has always succeeded
        res = bass_utils.run_bass_kernel_spmd(
            nc, in_maps, core_ids=list(range(NCORES))
        )
    global LAST_RESULTS
    LAST_RESULTS = res
    parts = np.stack([r["out_nd"] for r in res.results], axis=0)  # [8, 64, 66]
    tot = parts.sum(axis=0)
    return (tot[:, :C] / tot[:, C : C + 1]).astype(np.float32)


# revision 23
# speedup vs baseline: 1.0844x; 1.0592x over previous
"""Trainium2 Bass kernel for the CCM retrieval problem.

Reference computation (shapes: bs=64, N=1024, D=2048, H=128, C=65):
    z_x   = softmax(cos(all_f, emb)/T/sqrt(N))            [bs, N]
    hf    = head(all_f); hz = head(emb)                   [bs, H], [N, H]
    h1    = relu(BN(hf[b] @ A + b1 + hz[n] @ B))          [bs, N, H]
    y_zx  = softmax((h1 @ mix_w2 + mix_b2) @ clf_w + clf_b)  [bs, N, C]
    p_x   = softmax(sum_n cos(all_f, all_f)/T/sqrt(bs))   [bs]
    out   = z_x @ einsum('b,bnc->nc', p_x, y_zx)          [bs, C]

Device strategy: shard the queue axis N across 8 cores (128 rows each); bs
stays replicated so sum_x is core-local.  Host folds the BN affines into
weights, pre-multiplies mix_w2 @ clf_w (W2C) so the [bs,N,D] intermediate
never exists, pre-normalizes the embedding rows, and precomputes p_x.

Per core: input DMA is spread over four engine queues (sync/vector carry
etf halves, scalar/gpsimd carry w1h halves) so the ~1.3MB payload lands in
half the time; head-layer matmuls are emitted in chunk-arrival order.  The
mixer's first layer collapses to alpha[h,b] (64 cols) and beta[h,n] (128
cols); per b a fused relu(beta + alpha[:,b]) tensor_scalar produces the
logits stationary, spread across DVE (4x mode) / ScalarE / GpSimd one group
ahead of the PE.  exp on ScalarE; the softmax row-sums, reciprocal and
p_x-scale run per 8-b half on DVE so the e2 weighting (split DVE/GpSimd)
and the PSUM-accumulated b-sum start as early as possible.  Each core
returns [64, 66]: columns 0:65 are exp(z-score) @ sum_x partial numerators,
column 65 the z_x softmax denominator partial; the host sums partials over
cores and divides.
"""

import numpy as np
import ml_dtypes

import concourse.bass as bass
import concourse.tile as tile
from concourse import bacc, mybir
from concourse import bass_utils

F32 = mybir.dt.float32
BF16 = mybir.dt.bfloat16
F8 = mybir.dt.float8e4
AX = mybir.AxisListType
ALU = mybir.AluOpType
ACTF = mybir.ActivationFunctionType

T = 0.07
BN_EPS = 1e-5
BS, D, N, H, C = 64, 2048, 1024, 128, 65
NCORES = 8
NLOC = N // NCORES          # 128 queue rows per core
DCH = D // 128              # 16 contraction chunks
CP = 66                     # padded C stride (even -> 4B-aligned bf16 rows)
GRP = 16                    # b's per group (S/e2/accum granularity)
NG = BS // GRP              # 4 groups
HALF = 8                    # b's per 2-bank psum tile (4 per bank)

# tuning knobs --------------------------------------------------------------
# N_WARM_MM: junk matmuls before the head to ramp the PE p-state / HAM.
# SPREADS[g]: (n_dve, n_scalar, n_gpsimd) u-producers for group g; u's for
#   group g are emitted one group ahead of their logits matmuls.
# E2_CHUNKS: (engine, j0, j1) pieces of the e2 = e * w multiply.
import os as _os
N_WARM_MM = int(_os.environ.get("K_WARM", "8"))
_SP = _os.environ.get("K_SPREAD", "79")
SPREADS = ((9, 7, 0), (8, 8, 0),
           (int(_SP[0]), int(_SP[1:]) if len(_SP) > 2 else int(_SP[1]), 0),
           (int(_SP[0]), int(_SP[1:]) if len(_SP) > 2 else int(_SP[1]), 0))
_GPJ = int(_os.environ.get("K_GPJ", "10"))
E2_CHUNKS = (("v", 0, 8), ("v", 8, _GPJ), ("g", _GPJ, 16)) if _GPJ > 8 else (
    ("v", 0, 8), ("g", 8, 16))
WG_GPS = True
DMA_SPLIT4 = True


def _build(with_c0: bool):
    nc = bacc.Bacc("TRN2", target_bir_lowering=False, debug=False)

    d_etf = nc.dram_tensor("etf", [128, DCH * 96], BF16, kind="ExternalInput")
    d_w1h = nc.dram_tensor("w1h", [128, DCH * H // 2], BF16, kind="ExternalInput")
    d_pf = nc.dram_tensor("pf", [128, 130], F32, kind="ExternalInput")
    d_pb = nc.dram_tensor("pb", [128, 449], BF16, kind="ExternalInput")
    if with_c0:
        d_c0 = nc.dram_tensor("c0t", [1, C], BF16, kind="ExternalInput")
    d_out = nc.dram_tensor("out_nd", [BS, C + 1], F32, kind="ExternalOutput")

    with tile.TileContext(nc) as tc:
        with (
            tc.tile_pool(name="consts", bufs=1) as consts,
            tc.tile_pool(name="big", bufs=1) as bigp,
            tc.tile_pool(name="work", bufs=2) as work,
            tc.tile_pool(name="ubuf", bufs=32) as ubuf,
            tc.tile_pool(name="ebuf", bufs=3) as ebuf,
            tc.tile_pool(name="e2buf", bufs=2) as e2buf,
            tc.tile_pool(name="pbig", bufs=3, space="PSUM") as pbig,
            tc.tile_pool(name="phead", bufs=1, space="PSUM") as phead,
            tc.tile_pool(name="psmall", bufs=1, space="PSUM") as psmall,
        ):
            # warmup tiles: memsets first so the junk-fed PE can spin as
            # early as possible (p-state ramp + HAM un-throttle)
            warm = consts.tile([1, 1], F32)
            nc.gpsimd.memset(warm, 0.0)
            wl = consts.tile([128, 128], BF16)
            nc.gpsimd.memset(wl, 0.0)
            wr = consts.tile([128, 512], BF16)
            nc.gpsimd.memset(wr, 0.0)
            ones_col = consts.tile([128, 1], BF16)
            nc.gpsimd.memset(ones_col, 1.0)
            if with_c0:
                ones_row_bf = consts.tile([1, 128], BF16)
                nc.gpsimd.memset(ones_row_bf, 1.0)
            for _ in range(N_WARM_MM):
                pw = pbig.tile([128, 1024], F32, tag="pb")
                nc.tensor.matmul(pw[:, 0:512], wl, wr, start=True, stop=True)

            # ---- input DMAs over three engine queues ----
            pf = consts.tile([128, 130], F32)
            pb = consts.tile([128, 449], BF16)
            b1h_sb, cc_sb = pf[:, 0:1], pf[:, 1:2]
            nfs_b, px_b = pf[:, 2:66], pf[:, 66:130]
            wh2_sb, am_sb, bm_sb = pb[:, 0:128], pb[:, 128:256], pb[:, 256:384]
            w2c_sb = pb[:, 384:449]
            if with_c0:
                c0_sb = consts.tile([1, C], BF16)
                nc.sync.dma_start(out=c0_sb, in_=d_c0.ap())

            etf = bigp.tile([128, DCH, 192], F8)
            w1h_sb = bigp.tile([128, DCH, H], F8)
            etf_view = d_etf.ap().bitcast(F8).rearrange("p (i c) -> p i c", i=DCH)
            w1h_view = d_w1h.ap().bitcast(F8).rearrange("p (i h) -> p i h", i=DCH)
            if DMA_SPLIT4:
                # fp8 payload ~725KB: sync etf(0:10) then pf | scalar
                # w1h(0:10) then pb | gpsimd (SWDGE) etf(10:16)+w1h(10:16).
                # pf/pb ride last: their consumers (x1 bias, head2/ab
                # weights, w2c, nfs/px) all run after head chunk 0 anyway.
                for a, b in ((0, 2), (2, 6), (6, 10)):
                    sl = slice(a, b)
                    nc.sync.dma_start(out=etf[:, sl, :], in_=etf_view[:, sl, :])
                nc.sync.dma_start(out=pf, in_=d_pf.ap())
                for a, b in ((0, 2), (2, 6), (6, 10)):
                    sl = slice(a, b)
                    nc.scalar.dma_start(out=w1h_sb[:, sl, :], in_=w1h_view[:, sl, :])
                nc.scalar.dma_start(out=pb, in_=d_pb.ap())
                sl = slice(10, 16)
                nc.gpsimd.dma_start(out=etf[:, sl, :], in_=etf_view[:, sl, :])
                sl = slice(10, 16)
                nc.gpsimd.dma_start(out=w1h_sb[:, sl, :], in_=w1h_view[:, sl, :])
                chunk_order = list(range(DCH))
            else:
                nc.sync.dma_start(out=pf, in_=d_pf.ap())
                nc.scalar.dma_start(out=pb, in_=d_pb.ap())
                for a, b in ((0, 2), (2, 4), (4, 8), (8, 16)):
                    sl = slice(a, b)
                    nc.sync.dma_start(out=etf[:, sl, :], in_=etf_view[:, sl, :])
                    nc.scalar.dma_start(out=w1h_sb[:, sl, :], in_=w1h_view[:, sl, :])
                chunk_order = list(range(DCH))
            warm2 = consts.tile([1, 1], F32)
            nc.scalar.activation(warm2, warm, ACTF.Exp)

            # ---- head layer 1: X1 = relu(W1h.T @ [embT | all_fT] + b1h) ----
            xt = phead.tile([128, 192], F32, tag="ph")
            for k, i in enumerate(chunk_order):
                nc.tensor.matmul(
                    xt, w1h_sb[:, i, :], etf[:, i, :], start=(k == 0),
                    stop=(k == DCH - 1), skip_group_check=True,
                )
            x1 = work.tile([128, 192], BF16)
            nc.scalar.activation(
                x1[:, 0:128], xt[:, 0:128], ACTF.Relu, bias=b1h_sb,
                scale=1.0 / 256.0,
            )
            nc.scalar.activation(
                x1[:, 128:192], xt[:, 128:192], ACTF.Relu, bias=b1h_sb,
                scale=1.0 / 16.0,
            )
            # head layer 2 (head_b2 folded into cc)
            x2p = phead.tile([128, 192], F32, tag="ph")
            nc.tensor.matmul(x2p, wh2_sb, x1, skip_group_check=True)
            x2 = work.tile([128, 192], BF16)
            nc.scalar.copy(x2, x2p)
            hz2 = x2[:, 0:128]
            hf2 = x2[:, 128:192]
            # mixer layer 1 collapses: alpha[h, b] (+cc), beta[h, n]
            abp = phead.tile([128, 192], F32, tag="ph")
            nc.tensor.matmul(abp[:, 0:64], am_sb, hf2, skip_group_check=True)
            nc.tensor.matmul(abp[:, 64:192], bm_sb, hz2, skip_group_check=True)
            # z_x scores gate only ez / accum(0): first half fills the PE
            # while alpha/betaT resolve, second half goes after logits(0)
            slp = psmall.tile([NLOC, BS], F32, tag="ps")
            for k, i in enumerate(chunk_order[:8]):
                nc.tensor.matmul(
                    slp, etf[:, i, 0:128], etf[:, i, 128:192], start=(k == 0),
                    stop=False, skip_group_check=True,
                )
            alpha = work.tile([128, 64], F32)
            nc.scalar.activation(alpha, abp[:, 0:64], ACTF.Identity, bias=cc_sb)
            betaT = work.tile([128, 128], BF16)
            nc.vector.tensor_copy(betaT, abp[:, 64:192])

            e_tiles = [None] * NG
            e2_tiles = [None] * NG
            u_tiles = [[None] * GRP for _ in range(NG)]

            def emit_us(g, which="vsg", s_lo=0, s_hi=GRP):
                # u producers for group g, spread (dve, scalar, gpsimd);
                # `which` selects the engine subset to emit now (and for
                # scalar, the sub-block [s_lo, s_hi) of its u's) so each
                # engine gets its u work at the right point of its queue
                nv, ns, ng_ = SPREADS[g]
                s_seen = 0
                for jg in range(GRP):
                    b = GRP * g + jg
                    a_col = alpha[:, b : b + 1]
                    if jg < nv:
                        if "v" not in which:
                            continue
                        u = ubuf.tile([128, 128], BF16, tag="u", name="u")
                        nc.vector.tensor_scalar(
                            u, betaT, a_col, 0.0, op0=ALU.add, op1=ALU.max
                        )
                    elif jg < nv + ns:
                        si = s_seen
                        s_seen += 1
                        if "s" not in which or not (s_lo <= si < s_hi):
                            continue
                        u = ubuf.tile([128, 128], BF16, tag="u", name="u")
                        nc.scalar.activation(u, betaT, ACTF.Relu, bias=a_col)
                    else:
                        if "g" not in which:
                            continue
                        u = ubuf.tile([128, 128], BF16, tag="u", name="u")
                        nc.gpsimd.tensor_scalar(
                            u, betaT, a_col, 0.0, op0=ALU.add, op1=ALU.max
                        )
                    u_tiles[g][jg] = u

            def emit_front(g, halves=(0, 1)):
                # logits matmuls + per-half exp (u's made a group ahead)
                if 0 in halves:
                    e_g = ebuf.tile([128, GRP, CP], BF16, tag="e")
                    e_tiles[g] = e_g
                e_g = e_tiles[g]
                for h in halves:
                    pg = pbig.tile([128, 1024], F32, tag="pb")
                    for j in range(HALF):
                        jg = HALF * h + j
                        u = u_tiles[g][jg]
                        off = 512 * (j // 4) + C * (j % 4)
                        sl = pg[:, off : off + C]
                        if with_c0:
                            nc.tensor.matmul(
                                sl, ones_row_bf, c0_sb, start=True, stop=False,
                                skip_group_check=True,
                            )
                            nc.tensor.matmul(
                                sl, u, w2c_sb, start=False, stop=True,
                                skip_group_check=True,
                            )
                        else:
                            nc.tensor.matmul(
                                sl, u, w2c_sb, start=True, stop=True,
                                skip_group_check=True,
                            )
                    pg_v = pg.rearrange("p (u x) -> p u x", u=2)[:, :, 0 : 4 * C]
                    pg_v = pg_v.rearrange("p u (j c) -> p u j c", c=C)
                    eh = e_g[:, HALF * h : HALF * (h + 1), 0:C]
                    nc.scalar.activation(
                        eh.rearrange("p (u j) c -> p u j c", u=2), pg_v, ACTF.Exp
                    )

            def emit_back(g):
                # per-half softmax denominators + p_x scale on DVE; each e2
                # chunk is emitted as soon as its half's wg exists so the
                # accum matmuls can start early
                e_g = e_tiles[g]
                wg = work.tile([128, GRP], BF16, tag="wg", name="wg")

                def e2_chunk(eng, j0, j1):
                    veng = nc.vector if eng == "v" else nc.gpsimd
                    self_sl = slice(j0, j1)
                    wv = (
                        wg[:, self_sl]
                        .unsqueeze(2)
                        .broadcast_to([128, j1 - j0, C])
                    )
                    veng.tensor_tensor(
                        e2_tiles[g][:, self_sl, 0:C],
                        e_g[:, self_sl, 0:C],
                        wv,
                        op=ALU.mult,
                    )

                for h in range(2):
                    hs = slice(HALF * h, HALF * (h + 1))
                    bs0 = GRP * g + HALF * h
                    sg = work.tile([128, HALF], F32, tag=f"sg{h}", name="sg")
                    nc.vector.reduce_sum(sg, e_g[:, hs, 0:C], axis=AX.X)
                    rg = work.tile([128, HALF], F32, tag=f"rg{h}", name="rg")
                    nc.vector.reciprocal_approx_fast(rg, sg)
                    weng = nc.gpsimd if WG_GPS else nc.vector
                    weng.tensor_tensor(
                        wg[:, hs], rg, px_b[:, bs0 : bs0 + HALF], op=ALU.mult
                    )
                    for eng, j0, j1 in E2_CHUNKS:
                        if HALF * h <= j0 < HALF * (h + 1):
                            e2_chunk(eng, j0, j1)
                for eng, j0, j1 in E2_CHUNKS:
                    if not (0 <= j0 < GRP) or not any(
                        HALF * h <= j0 < HALF * (h + 1) for h in range(2)
                    ):
                        e2_chunk(eng, j0, j1)

            def emit_accum(g):
                e2_g = e2_tiles[g]
                for j in range(GRP):
                    b = GRP * g + j
                    nc.tensor.matmul(
                        onp[:, 0:C], ez, e2_g[:, j, 0:C],
                        start=(b == 0), stop=(b == BS - 1),
                        skip_group_check=True,
                    )

            # pipeline: u's one group ahead; back(g) = denom/e2 right after
            # front(g+1); accum(g) follows immediately (same period)
            emit_us(0)
            emit_us(1)
            emit_front(0)
            # remaining score chunks ride the PE behind logits(0); then the
            # ez row and z_x denominator column resolve well before accum(0)
            for k, i in enumerate(chunk_order[8:]):
                nc.tensor.matmul(
                    slp, etf[:, i, 0:128], etf[:, i, 128:192], start=False,
                    stop=(k == DCH - 9), skip_group_check=True,
                )
            t3 = work.tile([NLOC, BS], F32)
            nc.vector.tensor_tensor(t3, slp, nfs_b, op=ALU.mult)
            ez = work.tile([NLOC, BS], BF16)
            nc.scalar.activation(ez, t3, ACTF.Exp)
            onp = psmall.tile([BS, C + 1], F32, tag="ps")
            for g in range(1, NG):
                e2_tiles[g - 1] = e2buf.tile([128, GRP, CP], BF16, tag="e2", name="e2g")
                if g + 1 < NG:
                    emit_us(g + 1, "s")
                emit_front(g)
                emit_back(g - 1)
                if g == 1:
                    # z_x denominator column: needs only ez; placed here so
                    # the in-order PE queue never stalls on it
                    nc.tensor.matmul(onp[:, C : C + 1], ez, ones_col)
                emit_accum(g - 1)
                if g + 1 < NG:
                    emit_us(g + 1, "v")
                    emit_us(g + 1, "g")
            e2_tiles[NG - 1] = e2buf.tile([128, GRP, CP], BF16, tag="e2", name="e2g")
            emit_back(NG - 1)
            emit_accum(NG - 1)

            # ---- ship the partial result ----
            on_s = work.tile([BS, C + 1], F32)
            nc.scalar.copy(on_s, onp)
            nc.sync.dma_start(out=d_out.ap(), in_=on_s)

    nc.compile()
    return nc


_CACHE: dict = {}
LAST_RESULTS = None  # BassKernelResults of the most recent run (for profiling)


def _get_nc(with_c0: bool):
    if with_c0 not in _CACHE:
        _CACHE[with_c0] = _build(with_c0)
    return _CACHE[with_c0]


def kernel(
    all_f, embedding, all_y,
    head_w1, head_b1, head_g, head_beta, head_rm, head_rv, head_w2, head_b2,
    mix_w1, mix_b1, mix_g, mix_beta, mix_rm, mix_rv, mix_w2, mix_b2,
    clf_w, clf_b,
):
    f64 = np.float64
    bf16 = ml_dtypes.bfloat16
    sh = head_g.astype(f64) / np.sqrt(head_rv.astype(f64) + BN_EPS)
    th = head_beta.astype(f64) - head_rm.astype(f64) * sh
    w1h = head_w1.astype(f64) * sh[None, :]
    b1h = (head_b1.astype(f64) * sh + th).astype(np.float32)[:, None]
    sm = mix_g.astype(f64) / np.sqrt(mix_rv.astype(f64) + BN_EPS)
    tm = mix_beta.astype(f64) - mix_rm.astype(f64) * sm
    am = mix_w1[:H].astype(f64) * sm[None, :]
    bm = mix_w1[H:].astype(f64) * sm[None, :]
    cm = mix_b1.astype(f64) * sm + tm
    ca = (head_b2.astype(f64) @ am + cm).astype(np.float32)[:, None]
    cb = (head_b2.astype(f64) @ bm).astype(np.float32)[:, None]
    w2c = (mix_w2.astype(f64) @ clf_w.astype(f64)).astype(bf16)
    c0 = (mix_b2.astype(f64) @ clf_w.astype(f64) + clf_b.astype(f64)).astype(
        np.float32
    )
    with_c0 = bool(np.any(c0 != 0.0))

    af = np.ascontiguousarray(all_f, dtype=np.float32)
    emb = np.ascontiguousarray(embedding, dtype=np.float32)
    # input-side host prep: row norms folded into the bf16 payloads, p_x
    nf = 1.0 / np.sqrt((af.astype(f64) ** 2).sum(axis=1))           # [bs]
    nfs = (nf / (T * np.sqrt(N)) / 16.0).astype(np.float32)
    gscore = ((af @ af.T).astype(f64) * nf[:, None] * nf[None, :]).sum(axis=1)
    gscore = gscore / (T * np.sqrt(BS))
    pe = np.exp(gscore - gscore.max())
    px = pe / pe.sum()                                              # [bs]
    f8 = ml_dtypes.float8_e4m3
    aft = np.ascontiguousarray(af.T).astype(f8)                     # [D, bs]

    pack_bf = np.empty((128, 449), dtype=bf16)
    pack_bf[:, 0:128] = np.ascontiguousarray(head_w2).astype(bf16)
    pack_bf[:, 128:256] = am.astype(bf16)
    pack_bf[:, 256:384] = bm.astype(bf16)
    pack_bf[:, 384:449] = w2c
    pack_f = np.empty((128, 130), dtype=np.float32)
    pack_f[:, 0:1] = b1h
    pack_f[:, 1:2] = ca + cb
    pack_f[:, 2:66] = np.broadcast_to(nfs[None, :], (128, BS))
    pack_f[:, 66:130] = np.broadcast_to(px[None, :].astype(np.float32), (128, BS))
    # w1h repack: [128, DCH*H], row p holds chunks i (w1h[i*128+p, :])
    w1h_p = np.ascontiguousarray(
        (w1h * 16.0).reshape(DCH, 128, H).transpose(1, 0, 2).reshape(128, DCH * H)
    ).astype(f8)
    base = {"w1h": w1h_p.view(bf16), "pb": pack_bf, "pf": pack_f}
    if with_c0:
        base["c0t"] = c0[None, :].astype(bf16)

    ne = 1.0 / np.sqrt((emb.astype(f64) ** 2).sum(axis=1))          # [N]
    embn = (emb * ne[:, None].astype(np.float32)).astype(np.float64)

    in_maps = []
    for i in range(NCORES):
        shard = embn[i * NLOC : (i + 1) * NLOC]                     # [128, D]
        # etf packed [128, DCH*192]: row p, chunk i = [embT | afT] rows i*128+p
        etf3 = np.empty((128, DCH, 192), dtype=f8)
        shard_t = (shard.T * 16.0).reshape(DCH, 128, NLOC)          # [DCH,128,128]
        etf3[:, :, 0:128] = shard_t.transpose(1, 0, 2).astype(f8)
        etf3[:, :, 128:192] = aft.reshape(DCH, 128, BS).transpose(1, 0, 2)
        in_maps.append(dict(base, etf=etf3.reshape(128, DCH * 192).view(bf16)))

    nc = _get_nc(with_c0)
    try:
        res = bass_utils.run_bass_kernel_spmd(
            nc, in_maps, core_ids=list(range(NCORES))
        )
    except Exception:
        # fresh NEFFs occasionally hit a transient NRT exec fault on their
        # first dispatch; one retry has always succeeded
        res = bass_utils.run_bass_kernel_spmd(
            nc, in_maps, core_ids=list(range(NCORES))
        )
    global LAST_RESULTS
    LAST_RESULTS = res
    parts = np.stack([r["out_nd"] for r in res.results], axis=0)  # [8, 64, 66]
    tot = parts.sum(axis=0)
    return (tot[:, :C] / tot[:, C : C + 1]).astype(np.float32)
